# revision 122
# baseline (speedup 1.0000x reference)
"""Trainium2 Bass kernel v2 for nn_MoDBlock (mixture-of-depths block).

Per batch sequence b:
  scores = x_b @ w_router, computed exactly as (xf8 + r) @ w_router where
           xf8 = fp8(x) and r = bf16(x - xf8); the split halves the score
           DMA bytes while keeping ~1e-4 relative accuracy, far below the
           ~8e-4 top-k boundary gap, so the selected set matches f32 topk
  pos    = top-512 token positions (exact kth_largest threshold + gpsimd
           sparse_gather compaction), ascending
  tokens = x_b[pos] as fp8, gathered TRANSPOSED by gpsimd dma_gather
  causal 16-head attention over the 512 compacted tokens + w_proj
  layernorm + MLP (gelu-tanh)
  host: out[b, pos] += (partial_even + partial_odd) / 64

Sharding: 8 cores, core c = (batch b=c//2, half e=c%2). Scoring is split
across the pair: each core scores its half from HOST-pretransposed fp8 +
bf16-residual streams, both passes on the Tensor engine (replicated
weight columns), merged by an 8KB pair-AllGather into a p-major DRAM row
that loads straight into the kth_largest / scores16 layouts. Selection /
gather / proj / LN are computed redundantly by the pair; ATTENTION IS
HEAD-SPLIT — each core computes q/k/v and causal attention for only its
8 heads (via host-sliced wqkv columns), and the pair exchanges the fp8
oT halves in two pipelined AllGathers (shipped as int16 bytes) so proj
sees bit-identical inputs to the unsplit version. The MLP stays split by
hidden columns (w_fc cols / w_out rows) through the input weight data,
so the compiled program is identical on every core. Each core returns
its partial out-projection [512,1024] bf16 (x64 via the host-side weight
pre-scale); the host sums the pair, divides by 64, and scatter-adds into
x (the f32 residual stays exact on host).

Precision: weights are pre-scaled x64 and cast to fp8e4 (DoubleRow
matmuls: 2x PE throughput, half the weight DMA bytes); activations flow
bf16/fp8 with all matmul accumulation in f32 PSUM.

Attention layout: S^T[k,q] = (kT_h)^T @ qT_h per 128-key block, exp'd
directly into SBUF (bf16) so PV needs no P transposes; causal masking is
a tril multiply on the diagonal block; softmax row sums come from a
ones-column matmul alongside PV, normalization is a per-partition scalar
multiply, and the oT transposes run per-qb right behind each norm. The
LN row-sum rides the proj copies' accum_out.

DMA scheduling: the wire is FIFO and a queued DMA blocks its issuing
queue until the wire accepts it, so big weight loads are gated with tiny
dummy data-deps (wq on the merged scores, wk on the gather indices,
wv/wproj/wfc/wout on the gathered tokens) to slot them around the
latency-critical selection path, latency-critical small DMAs avoid
queues hosting big dispatches, and the exchange ladder runs on the
SP + Pool(SWDGE) queues to keep the Act queue free for attention exps.
HW constraints honored (the cost-model sim is laxer): GPSIMD never
touches PSUM, Act never writes fp8, DoubleRow Ldweights get >=128
columns, collective in/out APs are contiguous DRAM.
"""

import sys
from contextlib import ExitStack

sys.path.insert(0, "/opt/trn_rl_repo")

import numpy as np
import ml_dtypes

from concourse import bass, mybir, tile, bacc
from concourse.bass_utils import run_bass_kernel_spmd

BF16NP = ml_dtypes.bfloat16
F8NP = ml_dtypes.float8_e4m3
F32 = mybir.dt.float32
BF = mybir.dt.bfloat16
F8 = mybir.dt.float8e4
I32 = mybir.dt.int32
I16 = mybir.dt.int16
U32 = mybir.dt.uint32
AF = mybir.ActivationFunctionType
OP = mybir.AluOpType
DR = mybir.MatmulPerfMode.DoubleRow

D = 1024
S = 4096
B = 4
H = 16
HD = 64
K = 512
FCH = 2048           # fc hidden columns per core (4096 / 2)
WS = 64.0            # weight pre-scale folded into fp8 weights
WSI = 1.0 / WS


def build_program(n_cores=8, gelu_exact=False, collectives=True, debug=False):
    nc = bacc.Bacc(
        "TRN2", target_bir_lowering=False, debug=False, num_devices=n_cores
    )

    # ---- I/O ----
    xf8 = nc.dram_tensor("xf8", [S, D], F8, kind="ExternalInput")
    x8s = nc.dram_tensor("x8s", [4, 128, 4, 2, 512], F8, kind="ExternalInput")
    xrT = nc.dram_tensor("xrT", [4, 128, 8, 512], BF, kind="ExternalInput")
    w8drd = nc.dram_tensor("w8dr", [128, 4, 2, 128], F8,
                           kind="ExternalInput")
    w64d = nc.dram_tensor("w64", [128, 8, 128], BF, kind="ExternalInput")
    wqkv = nc.dram_tensor("wqkv", [128, 4, 2, 3 * 512], F8,
                          kind="ExternalInput")
    ot_out0 = nc.dram_tensor("ot_out0", [2, 128, K], I16)
    ot_out1 = nc.dram_tensor("ot_out1", [2, 128, K], I16)
    wproj = nc.dram_tensor("wproj", [128, 4, 2, D], F8, kind="ExternalInput")
    wfc = nc.dram_tensor("wfc", [128, 4, 2, FCH], F8, kind="ExternalInput")
    wout = nc.dram_tensor("wout", [128, 8, 2, D], F8, kind="ExternalInput")
    identbd = nc.dram_tensor("identb", [128, 128], BF, kind="ExternalInput")
    ones32d = nc.dram_tensor("ones32", [1, 16], F32, kind="ExternalInput")
    iota16d = nc.dram_tensor("iota16", [16, 256], F32, kind="ExternalInput")
    rep16d = nc.dram_tensor("rep16", [16, 128], F32, kind="ExternalInput")
    onesbd = nc.dram_tensor("onesb", [128, 8], BF, kind="ExternalInput")
    trilqd = nc.dram_tensor("trilq", [128, 128], BF, kind="ExternalInput")

    updp = nc.dram_tensor("updp", [128, 4, D], BF, kind="ExternalOutput")
    pos_out = nc.dram_tensor("pos_out", [16, 32], I32, kind="ExternalOutput")
    nf_out = nc.dram_tensor("nf_out", [1, 1], U32, kind="ExternalOutput")
    groups = [[i, i + 1] for i in range(0, n_cores, 2)]
    ag_out2 = nc.dram_tensor("ag_out2", [2, S // 2], F32)
    if debug:
        tokT_dbg = nc.dram_tensor("tokT_dbg", [128, 8, K], F8,
                                  kind="ExternalOutput")
        attn_dbg = nc.dram_tensor("attn_dbg", [128, 4, D], BF,
                                  kind="ExternalOutput")
        sc_dbg = nc.dram_tensor("sc_dbg", [128, 32], F32,
                                kind="ExternalOutput")
        s16_dbg = nc.dram_tensor("s16_dbg", [16, 256], F32,
                                 kind="ExternalOutput")

    with tile.TileContext(nc) as tc, ExitStack() as ctx:
        const = ctx.enter_context(tc.tile_pool(name="const", bufs=1))
        wp = ctx.enter_context(tc.tile_pool(name="wp", bufs=1))
        xsp = ctx.enter_context(tc.tile_pool(name="xsp", bufs=3))
        act = ctx.enter_context(tc.tile_pool(name="act", bufs=1))
        ptp = ctx.enter_context(tc.tile_pool(name="ptp", bufs=5))
        sml = ctx.enter_context(tc.tile_pool(name="sml", bufs=8))
        mm = ctx.enter_context(tc.tile_pool(name="mm", bufs=3, space="PSUM"))
        ov = ctx.enter_context(tc.tile_pool(name="ov", bufs=2, space="PSUM"))
        ovr = ctx.enter_context(tc.tile_pool(name="ovr", bufs=1, space="PSUM"))
        trp = ctx.enter_context(tc.tile_pool(name="trp", bufs=2, space="PSUM"))
        drp = ctx.enter_context(tc.tile_pool(name="drp", bufs=1, space="DRAM"))

        # ---- phase 1: router scores over this core's half of x ----
        # score*64 = x8*w8 + r'*w64 with x8 = fp8(x), w8 = fp8(64w),
        # r' = bf16(x - x8*(w8/w64)) host-computed: algebraically exact up
        # to the bf16 rounding of the small r' (~1e-4 of score, far below
        # the ~5e-2 scaled top-k boundary gap). Both passes run on the
        # otherwise-idle Tensor engine from HOST-pretransposed streams
        # (the score-pass "gather" had static indices, so the host does
        # the transpose): pass A fp8 DoubleRow, pass B bf16, both
        # accumulating into the same one-row PSUM. One tiny pair
        # AllGather merges the halves.
        w8dr_sb = const.tile([128, 4, 2, 128], F8, tag="w8dr")
        nc.scalar.dma_start(out=w8dr_sb[:], in_=w8drd[:, :, :, :])
        w64_sb = const.tile([128, 8, 128], BF, tag="w64")
        nc.scalar.dma_start(out=w64_sb[:], in_=w64d[:, :, :])
        scores = const.tile([128, 32], F32, tag="scores")
        scrow = const.tile([1, S // 2], F32, tag="scrow")
        ag_in2 = drp.tile([1, S // 2], F32, tag="agin2")
        for q in range(4):
            x8q = xsp.tile([128, 8, 512], F8, tag="x8", name=f"x8q{q}")
            x8v = x8q[:].rearrange("p c t -> p (c t)").rearrange(
                "p (g j t) -> p g j t", g=4, j=2)
            nc.sync.dma_start(out=x8v[:, :, :, :], in_=x8s[q, :, :, :, :])
            xrq = xsp.tile([128, 4, D], BF, tag="xr", name=f"xrq{q}")
            xrv = xrq[:].rearrange("p a d -> p (a d)").rearrange(
                "p (blk t) -> p blk t", blk=8)
            nc.sync.dma_start(out=xrv[:, 0:4, :], in_=xrT[q, :, 0:4, :])
            nc.sync.dma_start(out=xrv[:, 4:8, :], in_=xrT[q, :, 4:8, :])
            sq = ov.tile([128, 512], F32, tag="ov", name=f"sq{q}")
            for g in range(4):
                nc.tensor.matmul(
                    out=sq[:, :], lhsT=w8dr_sb[:, g, :, :],
                    rhs=x8v[:, g, :, :],
                    start=(g == 0), stop=False, perf_mode=DR,
                )
            for blk in range(8):
                nc.tensor.matmul(
                    out=sq[:, :], lhsT=w64_sb[:, blk, :],
                    rhs=xrv[:, blk, :],
                    start=False, stop=(blk == 7),
                )
            # scrow is kept p-major (col = p*16 + t for token t*128+p of
            # this half) so the post-collective loads are big-descriptor
            nc.scalar.activation(
                out=scrow[0:1, :].rearrange(
                    "a (p c) -> a c p", c=16)[:, q * 4:(q + 1) * 4, :],
                in_=sq[0:1, :].rearrange("a (t p) -> a t p", p=128),
                func=AF.Copy)
        nc.scalar.dma_start(out=ag_in2[0:1, :], in_=scrow[0:1, :])
        # ---- constants ----
        identb = const.tile([128, 128], BF, tag="identb")
        nc.scalar.dma_start(out=identb[:], in_=identbd[:, :])
        ones32 = const.tile([1, 16], F32, tag="ones32")
        nc.scalar.dma_start(out=ones32[:], in_=ones32d[:, :])
        iota16 = const.tile([16, 256], F32, tag="iota16")
        nc.scalar.dma_start(out=iota16[:], in_=iota16d[:, :])
        onesb = const.tile([128, 8], BF, tag="onesb")
        nc.scalar.dma_start(out=onesb[:], in_=onesbd[:, :])
        trilq = const.tile([128, 128], BF, tag="trilq")
        nc.scalar.dma_start(out=trilq[:], in_=trilqd[:, :])
        rep16 = const.tile([16, 128], F32, tag="rep16")
        nc.scalar.dma_start(out=rep16[:], in_=rep16d[:, :])

        # ---- one tiny pair AllGather (the only collective) ----
        if collectives:
            nc.gpsimd.collective_compute(
                "AllGather", OP.bypass, replica_groups=groups,
                ins=[ag_in2[:, :]], outs=[ag_out2[:, :]],
            )
        else:
            nc.sync.dma_start(out=ag_out2[0:1, :], in_=ag_in2[:, :])
            nc.scalar.dma_start(out=ag_out2[1:2, :], in_=ag_in2[:, :])
        # scores[p, h*16+t] = half h's token t*128+p (p-major source)
        nc.sync.dma_start(
            out=scores[:, :].rearrange("p (h c) -> p h c", h=2),
            in_=ag_out2[:, :].rearrange("h (p c) -> p h c", c=16),
        )
        # scores16[r, (h*16+tt)*8+u] = token h*2048+tt*128+u*16+r, loaded
        # straight off the gathered rows in parallel with `scores`
        scores16 = const.tile([16, 256], F32, tag="s16")
        for hh in range(2):
            heng = nc.scalar if hh == 0 else nc.sync
            heng.dma_start(
                out=scores16[:, hh * 128:(hh + 1) * 128].rearrange(
                    "r (tt u) -> r tt u", u=8),
                in_=ag_out2[hh:hh + 1, :].rearrange(
                    "a (u r tt) -> (a r) tt u", u=8, r=16),
            )

        # ---- phase 2: exact 512th-largest score + positions ----
        kv = const.tile([1, 2], F32, tag="kv")
        nc.gpsimd.kth_largest(out_ap=kv[:], in_ap=scores[:], n_per_lane=32,
                              k=510, quantile=1.0 - 510.5 / 4095.0)
        thr = const.tile([16, 1], F32, tag="thr16")
        nc.gpsimd.partition_broadcast(thr[:, :], kv[0:1, 1:2], channels=16)
        if debug:
            nc.scalar.dma_start(out=sc_dbg[:, :], in_=scores[:])
            nc.scalar.dma_start(out=s16_dbg[:, :], in_=scores16[:])
        # m2 in {0,2}; vals = (m2-1)*(iota+1): selected -> pos+1 (>0),
        # masked -> -(pos+1) (<0); the +1 bias is removed after compaction
        m16 = const.tile([16, 256], F32, tag="m16")
        nc.vector.tensor_scalar(
            out=m16[:], in0=scores16[:], scalar1=thr[0:16, :1], scalar2=2.0,
            op0=OP.is_ge, op1=OP.mult,
        )
        vals16 = const.tile([16, 256], F32, tag="v16")
        nc.vector.scalar_tensor_tensor(
            out=vals16[:], in0=m16[:], scalar=-1.0, in1=iota16[:],
            op0=OP.add, op1=OP.mult,
        )
        pos16f = const.tile([16, 32], F32, tag="p16f")
        nf_sb = const.tile([1, 1], U32, tag="nf")
        nc.gpsimd.sparse_gather(out=pos16f[:], in_=vals16[:],
                                num_found=nf_sb[:])
        repps = ov.tile([128, 512], F32, tag="ov", name="repps")
        nc.tensor.matmul(out=repps[:, 0:32], lhsT=rep16[:],
                         rhs=pos16f[:, 0:32], start=True, stop=True)
        idx16 = const.tile([128, 32], I16, tag="idx16")
        nc.vector.tensor_scalar_add(idx16[:], repps[:, 0:32], -1.0)
        pos16i = const.tile([16, 32], I32, tag="p16i")
        nc.vector.tensor_scalar_add(pos16i[:], pos16f[:], -1.0)
        nc.scalar.dma_start(out=pos_out[:, :], in_=pos16i[:])
        nc.scalar.dma_start(out=nf_out[:, :], in_=nf_sb[:])

        # ---- phase 3: transposed gather -> tokT fp8 [128, 8, 512] ----
        # 16-bit-granularity transpose of fp8 rows: partition p, group g
        # holds the byte pair d = 2*(g*128+p)+j at free offset t*2+j;
        # the host's wqkv row permutation compensates.
        tokT = act.tile([128, 8, K], F8, tag="tokT")
        nc.gpsimd.dma_gather(
            out_ap=tokT[:, :, :], in_ap=xf8[:, :], idxs_ap=idx16[:, :],
            num_idxs=K, num_idxs_reg=K, elem_size=D, transpose=True,
        )
        if debug:
            nc.sync.dma_start(out=tokT_dbg[:, :, :], in_=tokT[:, :, :])
        tokTg = tokT[:].rearrange("p c t -> p (c t)").rearrange(
            "p (g t j) -> p g j t", g=4, j=2)
        # stationary (Ldweights) operands may not use the byte-interleaved
        # dual-fp8 layout: repack for the v matmul's lhsT
        tokT2 = act.tile([128, 4, 2, K], F8, tag="tokT2")
        for g in range(4):
            eng = nc.vector if g % 2 == 0 else nc.gpsimd
            eng.tensor_copy(out=tokT2[:, g, :, :], in_=tokTg[:, g, :, :])

        # ---- weights: the wire is FIFO and a queued DMA blocks its issue
        # queue until the wire accepts it, so the big loads are gated with
        # dummy data-deps to slot them around the latency-critical
        # selection path: wq on the merged scores (wire-idle window), wk/wv
        # on pos16f (they land just before/after the token gather), the
        # rest on tokT. ----
        wqkv_sb = wp.tile([128, 4, 2, 3 * 512], F8, tag="wqkv")
        wproj_sb = wp.tile([128, 4, 2, D], F8, tag="wproj")
        wfc_sb = wp.tile([128, 4, 2, FCH], F8, tag="wfc")
        wout_sb = wp.tile([128, 8, 2, D], F8, tag="wout")
        dum = const.tile([1, 8], F32, tag="dum")
        nc.vector.tensor_copy(out=dum[0:1, 0:4], in_=scores[0:1, 28:32])
        nc.vector.tensor_copy(out=wqkv_sb[0:1, 0, 0, 0:16].bitcast(F32),
                              in_=dum[0:1, 0:4])
        nc.sync.dma_start(out=wqkv_sb[:, :, :, 0:512],
                          in_=wqkv[:, :, :, 0:512])
        dumk = const.tile([1, 8], I16, tag="dumk")
        nc.vector.tensor_copy(out=dumk[0:1, 0:4], in_=idx16[0:1, 0:4])
        nc.vector.tensor_copy(
            out=wqkv_sb[0:1, 0, 0, 512:512 + 8].bitcast(I16),
            in_=dumk[0:1, 0:4])
        nc.sync.dma_start(out=wqkv_sb[:, :, :, 512:1024],
                          in_=wqkv[:, :, :, 512:1024])
        dum2 = const.tile([1, 16], F8, tag="dum2")
        nc.vector.tensor_copy(out=dum2[0:1, 0:8], in_=tokT[0:1, 0, 0:8])
        nc.vector.tensor_copy(
            out=wqkv_sb[0:1, 0, 0, 1024:1024 + 8],
            in_=dum2[0:1, 0:8])
        nc.sync.dma_start(out=wqkv_sb[:, :, :, 1024:],
                          in_=wqkv[:, :, :, 1024:])
        nc.vector.tensor_copy(out=wproj_sb[0:1, 0, 0, 0:8],
                              in_=dum2[0:1, 0:8])
        nc.sync.dma_start(out=wproj_sb[:], in_=wproj[:, :, :, :])
        nc.vector.tensor_copy(out=wfc_sb[0:1, 0, 0, 0:8],
                              in_=dum2[0:1, 0:8])
        nc.sync.dma_start(out=wfc_sb[:], in_=wfc[:, :, :, :])
        nc.vector.tensor_copy(out=wout_sb[0:1, 0, 0, 0:8],
                              in_=dum2[0:1, 0:8])
        nc.sync.dma_start(out=wout_sb[:], in_=wout[:, :, :, :])

        # ---- phase 5: qkv (fp8 DoubleRow); q/k interleaved so head h's
        # operands complete early, v after ----
        qT = [None] * 4
        kT = [None] * 4
        for j in [0, 4, 1, 5, 2, 6, 3, 7]:
            qk = mm.tile([128, 512], F32, tag="mm", name=f"qkps{j}")
            for g in range(4):
                nc.tensor.matmul(
                    out=qk[:], lhsT=wqkv_sb[:, g, :, j * 128:(j + 1) * 128],
                    rhs=tokTg[:, g, :, :],
                    start=(g == 0), stop=(g == 3), perf_mode=DR,
                )
            t = act.tile([128, K], BF, tag=f"qkT{j}", name=f"qkT{j}")
            if j < 4:
                if j % 2 == 0:
                    nc.scalar.activation(out=t[:], in_=qk[:], func=AF.Copy,
                                         scale=0.125 * WSI)
                else:
                    nc.vector.tensor_scalar_mul(t[:], qk[:], 0.125 * WSI)
                qT[j] = t
            else:
                if j % 2 == 0:
                    nc.vector.tensor_scalar_mul(t[:], qk[:], WSI)
                else:
                    nc.scalar.activation(out=t[:], in_=qk[:], func=AF.Copy,
                                         scale=WSI)
                kT[j - 4] = t
        v_sb = act.tile([128, 4, 512], BF, tag="v")
        for c in range(4):
            vp = mm.tile([128, 512], F32, tag="mm", name=f"vps{c}")
            for g in range(4):
                nc.tensor.matmul(
                    out=vp[:],
                    lhsT=tokT2[:, g, :, c * 128:(c + 1) * 128],
                    rhs=wqkv_sb[:, g, :, 1024:1536],
                    start=(g == 0), stop=(g == 3), perf_mode=DR,
                )
            nc.vector.tensor_scalar_mul(v_sb[:, c, :], vp[:], WSI)

        # ---- phase 6: attention (this core's 8 heads; transposed
        # scores). The pair then AllGathers the fp8 oT halves, so proj
        # sees bit-identical inputs to the unsplit version. ----
        oT_loc = act.tile([128, 2, 2, K], F8, tag="oTl")
        ot_in0 = drp.tile([128, K], I16, tag="otin0")
        ot_in1 = drp.tile([128, K], I16, tag="otin1")
        oT_sb = act.tile([128, 4, 2, K], F8, tag="oT")
        of8_h = {}
        op_ps_h = {}
        for h in range(8):
            jt, prt = h // 2, 64 * (h % 2)
            qTh = qT[jt][prt:prt + 64, :]
            kTh = kT[jt][prt:prt + 64, :]
            PT = ptp.tile([128, 4, K], BF, tag="PT", name=f"PT{h}")
            for kb in range(4):
                qn = K - kb * 128
                st = mm.tile([128, 512], F32, tag="mm", name=f"st{h}_{kb}")
                nc.tensor.matmul(
                    out=st[:, :qn], lhsT=kTh[:, kb * 128:(kb + 1) * 128],
                    rhs=qTh[:, kb * 128:], start=True, stop=True,
                )
                nc.scalar.activation(out=PT[:, kb, :qn], in_=st[:, :qn],
                                     func=AF.Exp)
                meng = nc.gpsimd if (h % 4 == 3) else nc.vector
                meng.tensor_mul(out=PT[:, kb, :128], in0=PT[:, kb, :128],
                                in1=trilq[:])
            of8 = sml.tile([128, 4, 64], BF, tag="of8", name=f"of8{h}")
            of8_h[h] = of8
            if h % 2 == 0:
                op_ps = trp.tile([128, 1024], BF, tag="trp",
                                 name=f"otps{h}")
                op_ps_h[0] = op_ps
            else:
                op_ps = op_ps_h[0]
            for qb in range(4):
                o_ps = ov.tile([128, 512], F32, tag="ov", name=f"ops{h}_{qb}")
                r_ps = ovr.tile([128, 16], F32, tag="ovr", name=f"rps{h}_{qb}")
                for c in range(qb + 1):
                    nc.tensor.matmul(
                        out=o_ps[:, 0:64],
                        lhsT=PT[:, c, (qb - c) * 128:(qb - c + 1) * 128],
                        rhs=v_sb[:, c, h * 64:(h + 1) * 64],
                        start=(c == 0), stop=(c == qb),
                    )
                    nc.tensor.matmul(
                        out=r_ps[:, 0:1],
                        lhsT=PT[:, c, (qb - c) * 128:(qb - c + 1) * 128],
                        rhs=onesb[:, 0:1],
                        start=(c == 0), stop=(c == qb),
                    )
                rc = sml.tile([128, 1], F32, tag="rc", name=f"rc{h}_{qb}")
                nc.vector.reciprocal(rc[:], r_ps[:, 0:1])
                nc.vector.tensor_scalar(
                    out=of8[:, qb, :], in0=o_ps[:, 0:64], scalar1=rc[:, :1],
                    scalar2=None, op0=OP.mult,
                )
                nc.tensor.transpose(
                    out=op_ps[64 * (h % 2):64 * (h % 2) + 64,
                              qb * 128:(qb + 1) * 128],
                    in_=of8[:, qb, :],
                    identity=identb[:],
                )
            if h % 2 == 1:
                p = h // 2
                nc.vector.tensor_copy(out=oT_loc[:, p // 2, p % 2, :],
                                      in_=op_ps[:, 0:512])
                if p % 2 == 1:
                    pc = p // 2
                    ot_in = ot_in0 if pc == 0 else ot_in1
                    ot_o = ot_out0 if pc == 0 else ot_out1
                    nc.sync.dma_start(
                        out=ot_in[:, :].bitcast(F8).rearrange(
                            "p (j t) -> p j t", j=2),
                        in_=oT_loc[:, pc, :, :])
                    if collectives:
                        nc.gpsimd.collective_compute(
                            "AllGather", OP.bypass, replica_groups=groups,
                            ins=[ot_in[:, :]],
                            outs=[ot_o[:, :, :]],
                        )
                    else:
                        nc.gpsimd.dma_start(out=ot_o[0, :, :],
                                            in_=ot_in[:, :])
                        nc.sync.dma_start(out=ot_o[1, :, :],
                                          in_=ot_in[:, :])
                    # piece pc of both rows -> oT_sb groups pc and 2+pc
                    nc.gpsimd.dma_start(
                        out=oT_sb[:, pc, :, :],
                        in_=ot_o[0, :, :].bitcast(F8).rearrange(
                            "p (j t) -> p j t", j=2))
                    nc.sync.dma_start(
                        out=oT_sb[:, 2 + pc, :, :],
                        in_=ot_o[1, :, :].bitcast(F8).rearrange(
                            "p (j t) -> p j t", j=2))

        # ---- phase 7+8: proj (fp8 DoubleRow) -> attn rows bf16, with the
        # LN row-sum accumulated on the proj copies and a shortened
        # mean/var chain per tb ----
        attn_bf = act.tile([128, 4, D], BF, tag="attn")
        xin = act.tile([128, 4, D], BF, tag="xin")
        for tb in range(4):
            sm = sml.tile([128, 2], F32, tag="sm", name=f"sm{tb}")
            sq2 = sml.tile([128, 2], F32, tag="sq2", name=f"sq2{tb}")
            sqs = xsp.tile([128, 4, D], BF, tag="xr", name=f"sqs{tb}")
            for n in range(2):
                pp = mm.tile([128, 512], F32, tag="mm", name=f"pj{tb}_{n}")
                for gi, g in enumerate([0, 2, 1, 3]):
                    nc.tensor.matmul(
                        out=pp[:], lhsT=oT_sb[:, g, :, tb * 128:(tb + 1) * 128],
                        rhs=wproj_sb[:, g, :, n * 512:(n + 1) * 512],
                        start=(gi == 0), stop=(gi == 3), perf_mode=DR,
                    )
                half = attn_bf[:, tb, n * 512:(n + 1) * 512]
                if n == 0:
                    nc.vector.tensor_scalar(
                        out=half, in0=pp[:], scalar1=WSI, scalar2=0.0,
                        op0=OP.mult, op1=OP.add, accum_out=sm[:, n:n + 1])
                else:
                    nc.scalar.activation(
                        out=half, in_=pp[:], func=AF.Copy, scale=WSI,
                        accum_out=sm[:, n:n + 1])
                nc.vector.scalar_tensor_tensor(
                    out=sqs[:, n, 0:512], in0=half, scalar=0.0, in1=half,
                    op0=OP.add, op1=OP.mult, accum_out=sq2[:, n:n + 1],
                )
            at = attn_bf[:, tb, :]
            mu = sml.tile([128, 1], F32, tag="mu", name=f"mu{tb}")
            nc.vector.scalar_tensor_tensor(
                out=mu[:], in0=sm[:, 0:1], scalar=1.0, in1=sm[:, 1:2],
                op0=OP.mult, op1=OP.add)
            nc.vector.tensor_scalar_mul(mu[:], mu[:], 1.0 / D)
            mu2 = sml.tile([128, 1], F32, tag="mu2", name=f"mu2{tb}")
            nc.vector.tensor_scalar(
                out=mu2[:], in0=mu[:], scalar1=mu[:, 0:1], scalar2=-1e-5,
                op0=OP.mult, op1=OP.add)
            ssq = sml.tile([128, 1], F32, tag="ssq", name=f"ssq{tb}")
            nc.vector.scalar_tensor_tensor(
                out=ssq[:], in0=sq2[:, 0:1], scalar=1.0, in1=sq2[:, 1:2],
                op0=OP.mult, op1=OP.add)
            var = sml.tile([128, 1], F32, tag="var", name=f"var{tb}")
            nc.vector.scalar_tensor_tensor(
                out=var[:], in0=ssq[:], scalar=1.0 / D, in1=mu2[:],
                op0=OP.mult, op1=OP.subtract)
            sd = sml.tile([128, 1], F32, tag="sd", name=f"sd{tb}")
            nc.scalar.activation(out=sd[:], in_=var[:], func=AF.Sqrt)
            rr = sml.tile([128, 1], F32, tag="rr", name=f"rr{tb}")
            nc.vector.reciprocal(rr[:], sd[:])
            for nh in range(2):
                nc.vector.tensor_scalar(
                    out=xin[:, tb, nh * 512:(nh + 1) * 512],
                    in0=attn_bf[:, tb, nh * 512:(nh + 1) * 512],
                    scalar1=mu[:, :1], scalar2=rr[:, :1],
                    op0=OP.subtract, op1=OP.mult,
                )
        if debug:
            nc.sync.dma_start(out=attn_dbg[:, :, :], in_=attn_bf[:, :, :])

        # ---- phase 9: xiT fp8 [128, 4, 2, 512]; one [128,1024] PSUM
        # tile and one copy per g group ----
        xiT = act.tile([128, 4, 2, K], F8, tag="xiT")
        for g in range(4):
            xps = trp.tile([128, 1024], BF, tag="trp", name=f"xit{g}")
            for j in range(2):
                dc = g * 2 + j
                for tb in range(4):
                    nc.tensor.transpose(
                        out=xps[:, j * 512 + tb * 128:j * 512 + (tb + 1) * 128],
                        in_=xin[:, tb, dc * 128:(dc + 1) * 128],
                        identity=identb[:],
                    )
            if g % 2 == 0:
                nc.vector.tensor_copy(
                    out=xiT[:, g, :, :],
                    in_=xps[:].rearrange("p (j t) -> p j t", j=2))
            else:
                nc.scalar.activation(
                    out=xiT[:, g, :, :],
                    in_=xps[:].rearrange("p (j t) -> p j t", j=2),
                    func=AF.Copy)

        # ---- phase 10: fc + gelu (fp8 DoubleRow) ----
        hT = act.tile([128, 8, 2, K], F8, tag="hT")
        for f in range(16):
            fp = mm.tile([128, 512], F32, tag="mm", name=f"fc{f}")
            for g in range(4):
                nc.tensor.matmul(
                    out=fp[:], lhsT=wfc_sb[:, g, :, f * 128:(f + 1) * 128],
                    rhs=xiT[:, g, :, :],
                    start=(g == 0), stop=(g == 3), perf_mode=DR,
                )
            if not gelu_exact:
                hb = xsp.tile([128, 4, D], BF, tag="xr", name=f"hb{f}")
                nc.scalar.activation(out=hb[:, 0, 0:512], in_=fp[:],
                                     func=AF.Gelu_apprx_tanh, scale=WSI)
                heng = nc.vector if f % 2 == 0 else nc.gpsimd
                heng.tensor_copy(out=hT[:, f // 2, f % 2, :],
                                 in_=hb[:, 0, 0:512])
            else:
                # x*sigmoid(1.5957691*(x+0.044715*x^3)) via Exp+reciprocal
                hs = xsp.tile([128, 4, D], BF, tag="xr", name=f"gh{f}")
                x1 = hs[:, 0, 0:512]
                nc.vector.tensor_scalar_mul(x1, fp[:], WSI)
                h2 = hs[:, 1, 0:512]
                nc.vector.tensor_mul(out=h2, in0=x1, in1=x1)
                nc.vector.scalar_tensor_tensor(
                    out=h2, in0=h2, scalar=0.044715, in1=x1,
                    op0=OP.mult, op1=OP.mult,
                )
                nc.vector.tensor_add(out=h2, in0=h2, in1=x1)
                nc.scalar.activation(out=h2, in_=h2, func=AF.Exp,
                                     scale=-2.0 * 0.7978845608028654)
                nc.vector.tensor_scalar_add(h2, h2, 1.0)
                h3 = hs[:, 2, 0:512]
                with nc.allow_low_precision(reason="sigmoid denom, bf16 ok"):
                    nc.vector.reciprocal(h3, h2)
                nc.vector.tensor_mul(out=hT[:, f // 2, f % 2, :], in0=h3,
                                     in1=x1)

        # ---- phase 11: out partial (x64) -> updp bf16; per-half stores
        # so the final DMA tail is short, Act copy first / DVE last ----
        upd_sb = act.tile([128, 4, D], BF, tag="updp")
        for tb in range(4):
            for n in (1, 0):
                op_ps = mm.tile([128, 512], F32, tag="mm", name=f"ou{tb}_{n}")
                for g in range(8):
                    nc.tensor.matmul(
                        out=op_ps[:], lhsT=hT[:, g, :, tb * 128:(tb + 1) * 128],
                        rhs=wout_sb[:, g, :, n * 512:(n + 1) * 512],
                        start=(g == 0), stop=(g == 7), perf_mode=DR,
                    )
                nc.vector.tensor_copy(
                    out=upd_sb[:, tb, n * 512:(n + 1) * 512],
                    in_=op_ps[:])
                seng = nc.sync if n == 0 else nc.scalar
                seng.dma_start(out=updp[:, tb, n * 512:(n + 1) * 512],
                               in_=upd_sb[:, tb, n * 512:(n + 1) * 512])

    nc.compile()
    return nc


_CACHE = {}


def _get_program(n_cores=8):
    if n_cores not in _CACHE:
        _CACHE[n_cores] = build_program(n_cores)
    return _CACHE[n_cores]


def _prep_shared(inputs):
    """Host-side weight shuffles/casts (shared by all cores)."""
    w_router = np.asarray(inputs["w_router"], np.float32)
    w_qkv = np.asarray(inputs["w_qkv"], np.float32)
    w_proj = np.asarray(inputs["w_proj"], np.float32)
    w_fc = np.asarray(inputs["w_fc"], np.float32)
    w_out = np.asarray(inputs["w_out"], np.float32)

    w64 = w_router[:, 0].astype(np.float32) * WS
    w8 = w64.astype(F8NP).astype(np.float32)
    ratio = np.divide(w8, w64, out=np.zeros_like(w64), where=w64 != 0)
    # router weights in the standard DoubleRow k-map with 128 replicated
    # output columns: d = g*256 + j*128 + p
    w8dr = np.ascontiguousarray(np.broadcast_to(
        w8.reshape(4, 2, 128, 1).transpose(2, 0, 1, 3),
        (128, 4, 2, 128)).astype(F8NP))
    w64b = np.ascontiguousarray(np.broadcast_to(
        w64.reshape(8, 128).T[:, :, None], (128, 8, 128)).astype(BF16NP))
    identb = np.eye(128, dtype=BF16NP)
    iota16 = (np.arange(256)[None, :] * 16 + np.arange(16)[:, None]
              + 1).astype(np.float32)
    rep16 = np.zeros((16, 128), np.float32)
    for p in range(128):
        rep16[p % 16, p] = 1.0
    onesb = np.ones((128, 8), BF16NP)
    ar = np.arange(128)
    trilq = (ar[None, :] >= ar[:, None]).astype(BF16NP)

    p_ = np.arange(128)
    g_ = np.arange(4)
    j_ = np.arange(2)
    # gather-layout row map: tokT partition p, group g, sub j holds
    # x row d = 2*(g*128+p)+j
    dmap_gather = (2 * (g_[None, :, None] * 128 + p_[:, None, None])
                   + j_[None, None, :])
    wqkv_e = []
    for e in range(2):
        sl = np.concatenate(
            [w_qkv[:, o * D + e * 512:o * D + (e + 1) * 512]
             for o in range(3)], axis=1)
        wqkv_e.append(np.ascontiguousarray(
            (sl[dmap_gather.reshape(-1), :] * WS)
            .reshape(128, 4, 2, 3 * 512).astype(F8NP)))
    # standard DoubleRow k map: k row g*256 + j*128 + p
    dmap_std = (g_[None, :, None] * 256 + j_[None, None, :] * 128
                + p_[:, None, None])
    wproj_f8 = np.ascontiguousarray(
        (w_proj[dmap_std.reshape(-1), :] * WS).reshape(128, 4, 2, D)
        .astype(F8NP))
    halves = []
    g8 = np.arange(8)
    dmap8 = (g8[None, :, None] * 256 + j_[None, None, :] * 128
             + p_[:, None, None])
    for e in range(2):
        wfc_h = (w_fc[:, e * FCH:(e + 1) * FCH] * WS)
        wfc_f8 = np.ascontiguousarray(
            wfc_h[dmap_std.reshape(-1), :].reshape(128, 4, 2, FCH)
            .astype(F8NP))
        wout_h = (w_out[e * FCH:(e + 1) * FCH, :] * WS)
        wout_f8 = np.ascontiguousarray(
            wout_h[dmap8.reshape(-1), :].reshape(128, 8, 2, D).astype(F8NP))
        halves.append((wfc_f8, wout_f8))

    return dict(w8dr=w8dr, w64b=w64b, ratio=ratio,
                identb=identb, iota16=iota16,
                rep16=rep16, onesb=onesb, trilq=trilq,
                ones32=np.ones((1, 16), np.float32),
                wqkv_e=wqkv_e, wproj_f8=wproj_f8, halves=halves)


def make_in_maps(inputs, n_cores=8):
    x = np.asarray(inputs["x"], np.float32)
    sh = _prep_shared(inputs)
    xf8_all, x8s_all, xrT_all = [], {}, {}
    for b in range(B):
        xf8 = x[b].astype(F8NP)
        xrp = (x[b] - xf8.astype(np.float32) * sh["ratio"][None, :]
               ).astype(BF16NP)
        xf8_all.append(np.ascontiguousarray(xf8))
        for e in range(2):
            half8 = xf8[e * (S // 2):(e + 1) * (S // 2)]
            # x8s[q, p, g, j, s] = xf8[e*2048 + q*512 + s, g*256+j*128+p]
            x8s_all[b, e] = np.ascontiguousarray(
                half8.reshape(4, 512, 4, 2, 128).transpose(0, 4, 2, 3, 1))
            halfr = xrp[e * (S // 2):(e + 1) * (S // 2)]
            # xrT[q, p, blk, s] = r'[e*2048 + q*512 + s, blk*128+p]
            xrT_all[b, e] = np.ascontiguousarray(
                halfr.reshape(4, 512, 8, 128).transpose(0, 3, 2, 1))

    in_maps = []
    for c in range(n_cores):
        b, e = (c // 2) % B, c % 2
        wfc_f8, wout_f8 = sh["halves"][e]
        in_maps.append({
            "xf8": xf8_all[b],
            "x8s": x8s_all[b, e],
            "xrT": xrT_all[b, e],
            "w8dr": sh["w8dr"],
            "w64": sh["w64b"],
            "wqkv": sh["wqkv_e"][e],
            "wproj": sh["wproj_f8"],
            "wfc": wfc_f8,
            "wout": wout_f8,
            "identb": sh["identb"],
            "ones32": sh["ones32"],
            "iota16": sh["iota16"],
            "rep16": sh["rep16"],
            "onesb": sh["onesb"],
            "trilq": sh["trilq"],
        })
    return in_maps


def assemble_output(x, results):
    out = np.array(x, np.float32, copy=True)
    nb = len(results) // 2
    for b in range(nb):
        r0, r1 = results[2 * b], results[2 * b + 1]
        for r in (r0, r1):
            nf = int(np.asarray(r["nf_out"]).reshape(-1)[0])
            assert nf == K, f"batch {b}: expected {K} selected, got {nf}"
        pos = np.asarray(r0["pos_out"]).T.reshape(-1)     # [512] slot order
        u0 = np.asarray(r0["updp"]).astype(np.float32)    # [128, 4, 1024]
        u1 = np.asarray(r1["updp"]).astype(np.float32)
        part = (u0 + u1) * WSI
        rows = part.transpose(1, 0, 2).reshape(K, D)      # row s = tb*128+p
        out[b, pos] += rows
    return out


def kernel(**inputs):
    nc = _get_program(8)
    in_maps = make_in_maps(inputs, 8)
    res = run_bass_kernel_spmd(nc, in_maps, list(range(8))).results
    x = np.asarray(inputs["x"], np.float32)
    return assemble_output(x, res)


if __name__ == "__main__":
    nc = build_program(8)
    print("program built + compiled OK")



# revision 123
# speedup vs baseline: 1.0044x; 1.0044x over previous
"""Trainium2 Bass kernel v2 for nn_MoDBlock (mixture-of-depths block).

Per batch sequence b:
  scores = x_b @ w_router, computed exactly as (xf8 + r) @ w_router where
           xf8 = fp8(x) and r = bf16(x - xf8); the split halves the score
           DMA bytes while keeping ~1e-4 relative accuracy, far below the
           ~8e-4 top-k boundary gap, so the selected set matches f32 topk
  pos    = top-512 token positions (exact kth_largest threshold + gpsimd
           sparse_gather compaction), ascending
  tokens = x_b[pos] as fp8, gathered TRANSPOSED by gpsimd dma_gather
  causal 16-head attention over the 512 compacted tokens + w_proj
  layernorm + MLP (gelu-tanh)
  host: out[b, pos] += (partial_even + partial_odd) / 64

Sharding: 8 cores, core c = (batch b=c//2, half e=c%2). Scoring is split
across the pair: each core scores its half from HOST-pretransposed fp8 +
bf16-residual streams, both passes on the Tensor engine (replicated
weight columns), merged by an 8KB pair-AllGather into a p-major DRAM row
that loads straight into the kth_largest / scores16 layouts. Selection /
gather / proj / LN are computed redundantly by the pair; ATTENTION IS
HEAD-SPLIT — each core computes q/k/v and causal attention for only its
8 heads (via host-sliced wqkv columns), and the pair exchanges the fp8
oT halves in two pipelined AllGathers (shipped as int16 bytes) so proj
sees bit-identical inputs to the unsplit version. The MLP stays split by
hidden columns (w_fc cols / w_out rows) through the input weight data,
so the compiled program is identical on every core. Each core returns
its partial out-projection [512,1024] bf16 (x64 via the host-side weight
pre-scale); the host sums the pair, divides by 64, and scatter-adds into
x (the f32 residual stays exact on host).

Precision: weights are pre-scaled x64 and cast to fp8e4 (DoubleRow
matmuls: 2x PE throughput, half the weight DMA bytes); activations flow
bf16/fp8 with all matmul accumulation in f32 PSUM.

Attention layout: S^T[k,q] = (kT_h)^T @ qT_h per 128-key block, exp'd
directly into SBUF (bf16) so PV needs no P transposes; causal masking is
a tril multiply on the diagonal block; softmax row sums come from a
ones-column matmul alongside PV, normalization is a per-partition scalar
multiply, and the oT transposes run per-qb right behind each norm. The
LN row-sum rides the proj copies' accum_out.

DMA scheduling: the wire is FIFO and a queued DMA blocks its issuing
queue until the wire accepts it, so big weight loads are gated with tiny
dummy data-deps (wq on the merged scores, wk on the gather indices,
wv/wproj/wfc/wout on the gathered tokens) to slot them around the
latency-critical selection path, latency-critical small DMAs avoid
queues hosting big dispatches, and the exchange ladder runs on the
SP + Pool(SWDGE) queues to keep the Act queue free for attention exps.
HW constraints honored (the cost-model sim is laxer): GPSIMD never
touches PSUM, Act never writes fp8, DoubleRow Ldweights get >=128
columns, collective in/out APs are contiguous DRAM.
"""

import sys
from contextlib import ExitStack

sys.path.insert(0, "/opt/trn_rl_repo")

import numpy as np
import ml_dtypes

from concourse import bass, mybir, tile, bacc
from concourse.bass_utils import run_bass_kernel_spmd

BF16NP = ml_dtypes.bfloat16
F8NP = ml_dtypes.float8_e4m3
F32 = mybir.dt.float32
BF = mybir.dt.bfloat16
F8 = mybir.dt.float8e4
I32 = mybir.dt.int32
I16 = mybir.dt.int16
U32 = mybir.dt.uint32
AF = mybir.ActivationFunctionType
OP = mybir.AluOpType
DR = mybir.MatmulPerfMode.DoubleRow

D = 1024
S = 4096
B = 4
H = 16
HD = 64
K = 512
FCH = 2048           # fc hidden columns per core (4096 / 2)
WS = 64.0            # weight pre-scale folded into fp8 weights
WSI = 1.0 / WS


def build_program(n_cores=8, gelu_exact=False, collectives=True, debug=False):
    nc = bacc.Bacc(
        "TRN2", target_bir_lowering=False, debug=False, num_devices=n_cores
    )

    # ---- I/O ----
    xf8 = nc.dram_tensor("xf8", [S, D], F8, kind="ExternalInput")
    x8s = nc.dram_tensor("x8s", [4, 128, 4, 2, 512], F8, kind="ExternalInput")
    xrT = nc.dram_tensor("xrT", [4, 128, 8, 512], BF, kind="ExternalInput")
    w8drd = nc.dram_tensor("w8dr", [128, 4, 2, 32], F8,
                           kind="ExternalInput")
    w64d = nc.dram_tensor("w64", [128, 8, 32], BF, kind="ExternalInput")
    wqkv = nc.dram_tensor("wqkv", [128, 4, 2, 3 * 512], F8,
                          kind="ExternalInput")
    ot_out0 = nc.dram_tensor("ot_out0", [2, 128, K], I16)
    ot_out1 = nc.dram_tensor("ot_out1", [2, 128, K], I16)
    wproj = nc.dram_tensor("wproj", [128, 4, 2, D], F8, kind="ExternalInput")
    wfc = nc.dram_tensor("wfc", [128, 4, 2, FCH], F8, kind="ExternalInput")
    wout = nc.dram_tensor("wout", [128, 8, 2, D], F8, kind="ExternalInput")
    identbd = nc.dram_tensor("identb", [128, 128], BF, kind="ExternalInput")
    ones32d = nc.dram_tensor("ones32", [1, 16], F32, kind="ExternalInput")
    iota16d = nc.dram_tensor("iota16", [16, 256], F32, kind="ExternalInput")
    rep16d = nc.dram_tensor("rep16", [16, 128], F32, kind="ExternalInput")
    onesbd = nc.dram_tensor("onesb", [128, 8], BF, kind="ExternalInput")
    trilqd = nc.dram_tensor("trilq", [128, 128], BF, kind="ExternalInput")

    updp = nc.dram_tensor("updp", [128, 4, D], BF, kind="ExternalOutput")
    pos_out = nc.dram_tensor("pos_out", [16, 32], I32, kind="ExternalOutput")
    nf_out = nc.dram_tensor("nf_out", [1, 1], U32, kind="ExternalOutput")
    groups = [[i, i + 1] for i in range(0, n_cores, 2)]
    ag_out2 = nc.dram_tensor("ag_out2", [2, S // 2], F32)
    if debug:
        tokT_dbg = nc.dram_tensor("tokT_dbg", [128, 8, K], F8,
                                  kind="ExternalOutput")
        attn_dbg = nc.dram_tensor("attn_dbg", [128, 4, D], BF,
                                  kind="ExternalOutput")
        sc_dbg = nc.dram_tensor("sc_dbg", [128, 32], F32,
                                kind="ExternalOutput")
        s16_dbg = nc.dram_tensor("s16_dbg", [16, 256], F32,
                                 kind="ExternalOutput")

    with tile.TileContext(nc) as tc, ExitStack() as ctx:
        const = ctx.enter_context(tc.tile_pool(name="const", bufs=1))
        wp = ctx.enter_context(tc.tile_pool(name="wp", bufs=1))
        xsp = ctx.enter_context(tc.tile_pool(name="xsp", bufs=3))
        act = ctx.enter_context(tc.tile_pool(name="act", bufs=1))
        ptp = ctx.enter_context(tc.tile_pool(name="ptp", bufs=5))
        sml = ctx.enter_context(tc.tile_pool(name="sml", bufs=8))
        mm = ctx.enter_context(tc.tile_pool(name="mm", bufs=3, space="PSUM"))
        ov = ctx.enter_context(tc.tile_pool(name="ov", bufs=2, space="PSUM"))
        ovr = ctx.enter_context(tc.tile_pool(name="ovr", bufs=1, space="PSUM"))
        trp = ctx.enter_context(tc.tile_pool(name="trp", bufs=2, space="PSUM"))
        drp = ctx.enter_context(tc.tile_pool(name="drp", bufs=1, space="DRAM"))

        # ---- phase 1: router scores over this core's half of x ----
        # score*64 = x8*w8 + r'*w64 with x8 = fp8(x), w8 = fp8(64w),
        # r' = bf16(x - x8*(w8/w64)) host-computed: algebraically exact up
        # to the bf16 rounding of the small r' (~1e-4 of score, far below
        # the ~5e-2 scaled top-k boundary gap). Both passes run on the
        # otherwise-idle Tensor engine from HOST-pretransposed streams
        # (the score-pass "gather" had static indices, so the host does
        # the transpose): pass A fp8 DoubleRow, pass B bf16, both
        # accumulating into the same one-row PSUM. One tiny pair
        # AllGather merges the halves.
        w8dr_sb = const.tile([128, 4, 2, 32], F8, tag="w8dr")
        nc.scalar.dma_start(out=w8dr_sb[:], in_=w8drd[:, :, :, :])
        w64_sb = const.tile([128, 8, 32], BF, tag="w64")
        nc.scalar.dma_start(out=w64_sb[:], in_=w64d[:, :, :])
        scores = const.tile([128, 32], F32, tag="scores")
        scrow = const.tile([1, S // 2], F32, tag="scrow")
        ag_in2 = drp.tile([1, S // 2], F32, tag="agin2")
        for q in range(4):
            x8q = xsp.tile([128, 8, 512], F8, tag="x8", name=f"x8q{q}")
            x8v = x8q[:].rearrange("p c t -> p (c t)").rearrange(
                "p (g j t) -> p g j t", g=4, j=2)
            nc.sync.dma_start(out=x8v[:, :, :, :], in_=x8s[q, :, :, :, :])
            xrq = xsp.tile([128, 4, D], BF, tag="xr", name=f"xrq{q}")
            xrv = xrq[:].rearrange("p a d -> p (a d)").rearrange(
                "p (blk t) -> p blk t", blk=8)
            nc.sync.dma_start(out=xrv[:, 0:4, :], in_=xrT[q, :, 0:4, :])
            nc.sync.dma_start(out=xrv[:, 4:8, :], in_=xrT[q, :, 4:8, :])
            sq = ov.tile([128, 512], F32, tag="ov", name=f"sq{q}")
            for g in range(4):
                nc.tensor.matmul(
                    out=sq[0:32, :], lhsT=w8dr_sb[:, g, :, :],
                    rhs=x8v[:, g, :, :],
                    start=(g == 0), stop=False, perf_mode=DR,
                )
            for blk in range(8):
                nc.tensor.matmul(
                    out=sq[0:32, :], lhsT=w64_sb[:, blk, :],
                    rhs=xrv[:, blk, :],
                    start=False, stop=(blk == 7),
                )
            # scrow is kept p-major (col = p*16 + t for token t*128+p of
            # this half) so the post-collective loads are big-descriptor
            nc.scalar.activation(
                out=scrow[0:1, :].rearrange(
                    "a (p c) -> a c p", c=16)[:, q * 4:(q + 1) * 4, :],
                in_=sq[0:1, :].rearrange("a (t p) -> a t p", p=128),
                func=AF.Copy)
        nc.scalar.dma_start(out=ag_in2[0:1, :], in_=scrow[0:1, :])
        # ---- constants ----
        identb = const.tile([128, 128], BF, tag="identb")
        nc.scalar.dma_start(out=identb[:], in_=identbd[:, :])
        ones32 = const.tile([1, 16], F32, tag="ones32")
        nc.scalar.dma_start(out=ones32[:], in_=ones32d[:, :])
        iota16 = const.tile([16, 256], F32, tag="iota16")
        nc.scalar.dma_start(out=iota16[:], in_=iota16d[:, :])
        onesb = const.tile([128, 8], BF, tag="onesb")
        nc.scalar.dma_start(out=onesb[:], in_=onesbd[:, :])
        trilq = const.tile([128, 128], BF, tag="trilq")
        nc.scalar.dma_start(out=trilq[:], in_=trilqd[:, :])
        rep16 = const.tile([16, 128], F32, tag="rep16")
        nc.scalar.dma_start(out=rep16[:], in_=rep16d[:, :])

        # ---- one tiny pair AllGather (the only collective) ----
        if collectives:
            nc.gpsimd.collective_compute(
                "AllGather", OP.bypass, replica_groups=groups,
                ins=[ag_in2[:, :]], outs=[ag_out2[:, :]],
            )
        else:
            nc.sync.dma_start(out=ag_out2[0:1, :], in_=ag_in2[:, :])
            nc.scalar.dma_start(out=ag_out2[1:2, :], in_=ag_in2[:, :])
        # scores[p, h*16+t] = half h's token t*128+p (p-major source)
        nc.sync.dma_start(
            out=scores[:, :].rearrange("p (h c) -> p h c", h=2),
            in_=ag_out2[:, :].rearrange("h (p c) -> p h c", c=16),
        )
        # scores16[r, (h*16+tt)*8+u] = token h*2048+tt*128+u*16+r, loaded
        # straight off the gathered rows in parallel with `scores`
        scores16 = const.tile([16, 256], F32, tag="s16")
        for hh in range(2):
            heng = nc.scalar if hh == 0 else nc.sync
            heng.dma_start(
                out=scores16[:, hh * 128:(hh + 1) * 128].rearrange(
                    "r (tt u) -> r tt u", u=8),
                in_=ag_out2[hh:hh + 1, :].rearrange(
                    "a (u r tt) -> (a r) tt u", u=8, r=16),
            )

        # ---- phase 2: exact 512th-largest score + positions ----
        kv = const.tile([1, 2], F32, tag="kv")
        nc.gpsimd.kth_largest(out_ap=kv[:], in_ap=scores[:], n_per_lane=32,
                              k=510, quantile=1.0 - 510.5 / 4095.0)
        thr = const.tile([16, 1], F32, tag="thr16")
        nc.gpsimd.partition_broadcast(thr[:, :], kv[0:1, 1:2], channels=16)
        if debug:
            nc.scalar.dma_start(out=sc_dbg[:, :], in_=scores[:])
            nc.scalar.dma_start(out=s16_dbg[:, :], in_=scores16[:])
        # m2 in {0,2}; vals = (m2-1)*(iota+1): selected -> pos+1 (>0),
        # masked -> -(pos+1) (<0); the +1 bias is removed after compaction
        m16 = const.tile([16, 256], F32, tag="m16")
        nc.vector.tensor_scalar(
            out=m16[:], in0=scores16[:], scalar1=thr[0:16, :1], scalar2=2.0,
            op0=OP.is_ge, op1=OP.mult,
        )
        vals16 = const.tile([16, 256], F32, tag="v16")
        nc.vector.scalar_tensor_tensor(
            out=vals16[:], in0=m16[:], scalar=-1.0, in1=iota16[:],
            op0=OP.add, op1=OP.mult,
        )
        pos16f = const.tile([16, 32], F32, tag="p16f")
        nf_sb = const.tile([1, 1], U32, tag="nf")
        nc.gpsimd.sparse_gather(out=pos16f[:], in_=vals16[:],
                                num_found=nf_sb[:])
        repps = ov.tile([128, 512], F32, tag="ov", name="repps")
        nc.tensor.matmul(out=repps[:, 0:32], lhsT=rep16[:],
                         rhs=pos16f[:, 0:32], start=True, stop=True)
        idx16 = const.tile([128, 32], I16, tag="idx16")
        nc.vector.tensor_scalar_add(idx16[:], repps[:, 0:32], -1.0)
        pos16i = const.tile([16, 32], I32, tag="p16i")
        nc.vector.tensor_scalar_add(pos16i[:], pos16f[:], -1.0)
        nc.scalar.dma_start(out=pos_out[:, :], in_=pos16i[:])
        nc.scalar.dma_start(out=nf_out[:, :], in_=nf_sb[:])

        # ---- phase 3: transposed gather -> tokT fp8 [128, 8, 512] ----
        # 16-bit-granularity transpose of fp8 rows: partition p, group g
        # holds the byte pair d = 2*(g*128+p)+j at free offset t*2+j;
        # the host's wqkv row permutation compensates.
        tokT = act.tile([128, 8, K], F8, tag="tokT")
        nc.gpsimd.dma_gather(
            out_ap=tokT[:, :, :], in_ap=xf8[:, :], idxs_ap=idx16[:, :],
            num_idxs=K, num_idxs_reg=K, elem_size=D, transpose=True,
        )
        if debug:
            nc.sync.dma_start(out=tokT_dbg[:, :, :], in_=tokT[:, :, :])
        tokTg = tokT[:].rearrange("p c t -> p (c t)").rearrange(
            "p (g t j) -> p g j t", g=4, j=2)
        # stationary (Ldweights) operands may not use the byte-interleaved
        # dual-fp8 layout: repack for the v matmul's lhsT
        tokT2 = act.tile([128, 4, 2, K], F8, tag="tokT2")
        for g in range(4):
            eng = nc.vector if g % 2 == 0 else nc.gpsimd
            eng.tensor_copy(out=tokT2[:, g, :, :], in_=tokTg[:, g, :, :])

        # ---- weights: the wire is FIFO and a queued DMA blocks its issue
        # queue until the wire accepts it, so the big loads are gated with
        # dummy data-deps to slot them around the latency-critical
        # selection path: wq on the merged scores (wire-idle window), wk/wv
        # on pos16f (they land just before/after the token gather), the
        # rest on tokT. ----
        wqkv_sb = wp.tile([128, 4, 2, 3 * 512], F8, tag="wqkv")
        wproj_sb = wp.tile([128, 4, 2, D], F8, tag="wproj")
        wfc_sb = wp.tile([128, 4, 2, FCH], F8, tag="wfc")
        wout_sb = wp.tile([128, 8, 2, D], F8, tag="wout")
        dum = const.tile([1, 8], F32, tag="dum")
        nc.vector.tensor_copy(out=dum[0:1, 0:4], in_=scores[0:1, 28:32])
        nc.vector.tensor_copy(out=wqkv_sb[0:1, 0, 0, 0:16].bitcast(F32),
                              in_=dum[0:1, 0:4])
        nc.sync.dma_start(out=wqkv_sb[:, :, :, 0:512],
                          in_=wqkv[:, :, :, 0:512])
        dumk = const.tile([1, 8], I16, tag="dumk")
        nc.vector.tensor_copy(out=dumk[0:1, 0:4], in_=idx16[0:1, 0:4])
        nc.vector.tensor_copy(
            out=wqkv_sb[0:1, 0, 0, 512:512 + 8].bitcast(I16),
            in_=dumk[0:1, 0:4])
        nc.sync.dma_start(out=wqkv_sb[:, :, :, 512:1024],
                          in_=wqkv[:, :, :, 512:1024])
        dum2 = const.tile([1, 16], F8, tag="dum2")
        nc.vector.tensor_copy(out=dum2[0:1, 0:8], in_=tokT[0:1, 0, 0:8])
        nc.vector.tensor_copy(
            out=wqkv_sb[0:1, 0, 0, 1024:1024 + 8],
            in_=dum2[0:1, 0:8])
        nc.sync.dma_start(out=wqkv_sb[:, :, :, 1024:],
                          in_=wqkv[:, :, :, 1024:])
        nc.vector.tensor_copy(out=wproj_sb[0:1, 0, 0, 0:8],
                              in_=dum2[0:1, 0:8])
        nc.sync.dma_start(out=wproj_sb[:], in_=wproj[:, :, :, :])
        nc.vector.tensor_copy(out=wfc_sb[0:1, 0, 0, 0:8],
                              in_=dum2[0:1, 0:8])
        nc.sync.dma_start(out=wfc_sb[:], in_=wfc[:, :, :, :])
        nc.vector.tensor_copy(out=wout_sb[0:1, 0, 0, 0:8],
                              in_=dum2[0:1, 0:8])
        nc.sync.dma_start(out=wout_sb[:], in_=wout[:, :, :, :])

        # ---- phase 5: qkv (fp8 DoubleRow); q/k interleaved so head h's
        # operands complete early, v after ----
        qT = [None] * 4
        kT = [None] * 4
        for j in [0, 4, 1, 5, 2, 6, 3, 7]:
            qk = mm.tile([128, 512], F32, tag="mm", name=f"qkps{j}")
            for g in range(4):
                nc.tensor.matmul(
                    out=qk[:], lhsT=wqkv_sb[:, g, :, j * 128:(j + 1) * 128],
                    rhs=tokTg[:, g, :, :],
                    start=(g == 0), stop=(g == 3), perf_mode=DR,
                )
            t = act.tile([128, K], BF, tag=f"qkT{j}", name=f"qkT{j}")
            if j < 4:
                if j % 2 == 0:
                    nc.scalar.activation(out=t[:], in_=qk[:], func=AF.Copy,
                                         scale=0.125 * WSI)
                else:
                    nc.vector.tensor_scalar_mul(t[:], qk[:], 0.125 * WSI)
                qT[j] = t
            else:
                if j % 2 == 0:
                    nc.vector.tensor_scalar_mul(t[:], qk[:], WSI)
                else:
                    nc.scalar.activation(out=t[:], in_=qk[:], func=AF.Copy,
                                         scale=WSI)
                kT[j - 4] = t
        v_sb = act.tile([128, 4, 512], BF, tag="v")
        for c in range(4):
            vp = mm.tile([128, 512], F32, tag="mm", name=f"vps{c}")
            for g in range(4):
                nc.tensor.matmul(
                    out=vp[:],
                    lhsT=tokT2[:, g, :, c * 128:(c + 1) * 128],
                    rhs=wqkv_sb[:, g, :, 1024:1536],
                    start=(g == 0), stop=(g == 3), perf_mode=DR,
                )
            nc.vector.tensor_scalar_mul(v_sb[:, c, :], vp[:], WSI)

        # ---- phase 6: attention (this core's 8 heads; transposed
        # scores). The pair then AllGathers the fp8 oT halves, so proj
        # sees bit-identical inputs to the unsplit version. ----
        oT_loc = act.tile([128, 2, 2, K], F8, tag="oTl")
        ot_in0 = drp.tile([128, K], I16, tag="otin0")
        ot_in1 = drp.tile([128, K], I16, tag="otin1")
        oT_sb = act.tile([128, 4, 2, K], F8, tag="oT")
        of8_h = {}
        op_ps_h = {}
        for h in range(8):
            jt, prt = h // 2, 64 * (h % 2)
            qTh = qT[jt][prt:prt + 64, :]
            kTh = kT[jt][prt:prt + 64, :]
            PT = ptp.tile([128, 4, K], BF, tag="PT", name=f"PT{h}")
            for kb in range(4):
                qn = K - kb * 128
                st = mm.tile([128, 512], F32, tag="mm", name=f"st{h}_{kb}")
                nc.tensor.matmul(
                    out=st[:, :qn], lhsT=kTh[:, kb * 128:(kb + 1) * 128],
                    rhs=qTh[:, kb * 128:], start=True, stop=True,
                )
                nc.scalar.activation(out=PT[:, kb, :qn], in_=st[:, :qn],
                                     func=AF.Exp)
                meng = nc.gpsimd if (h % 4 == 3) else nc.vector
                meng.tensor_mul(out=PT[:, kb, :128], in0=PT[:, kb, :128],
                                in1=trilq[:])
            of8 = sml.tile([128, 4, 64], BF, tag="of8", name=f"of8{h}")
            of8_h[h] = of8
            if h % 2 == 0:
                op_ps = trp.tile([128, 1024], BF, tag="trp",
                                 name=f"otps{h}")
                op_ps_h[0] = op_ps
            else:
                op_ps = op_ps_h[0]
            for qb in range(4):
                o_ps = ov.tile([128, 512], F32, tag="ov", name=f"ops{h}_{qb}")
                r_ps = ovr.tile([128, 16], F32, tag="ovr", name=f"rps{h}_{qb}")
                for c in range(qb + 1):
                    nc.tensor.matmul(
                        out=o_ps[:, 0:64],
                        lhsT=PT[:, c, (qb - c) * 128:(qb - c + 1) * 128],
                        rhs=v_sb[:, c, h * 64:(h + 1) * 64],
                        start=(c == 0), stop=(c == qb),
                    )
                    nc.tensor.matmul(
                        out=r_ps[:, 0:1],
                        lhsT=PT[:, c, (qb - c) * 128:(qb - c + 1) * 128],
                        rhs=onesb[:, 0:1],
                        start=(c == 0), stop=(c == qb),
                    )
                rc = sml.tile([128, 1], F32, tag="rc", name=f"rc{h}_{qb}")
                nc.vector.reciprocal(rc[:], r_ps[:, 0:1])
                nc.vector.tensor_scalar(
                    out=of8[:, qb, :], in0=o_ps[:, 0:64], scalar1=rc[:, :1],
                    scalar2=None, op0=OP.mult,
                )
                nc.tensor.transpose(
                    out=op_ps[64 * (h % 2):64 * (h % 2) + 64,
                              qb * 128:(qb + 1) * 128],
                    in_=of8[:, qb, :],
                    identity=identb[:],
                )
            if h % 2 == 1:
                p = h // 2
                nc.vector.tensor_copy(out=oT_loc[:, p // 2, p % 2, :],
                                      in_=op_ps[:, 0:512])
                if p % 2 == 1:
                    pc = p // 2
                    ot_in = ot_in0 if pc == 0 else ot_in1
                    ot_o = ot_out0 if pc == 0 else ot_out1
                    nc.sync.dma_start(
                        out=ot_in[:, :].bitcast(F8).rearrange(
                            "p (j t) -> p j t", j=2),
                        in_=oT_loc[:, pc, :, :])
                    if collectives:
                        nc.gpsimd.collective_compute(
                            "AllGather", OP.bypass, replica_groups=groups,
                            ins=[ot_in[:, :]],
                            outs=[ot_o[:, :, :]],
                        )
                    else:
                        nc.gpsimd.dma_start(out=ot_o[0, :, :],
                                            in_=ot_in[:, :])
                        nc.sync.dma_start(out=ot_o[1, :, :],
                                          in_=ot_in[:, :])
                    # piece pc of both rows -> oT_sb groups pc and 2+pc
                    nc.gpsimd.dma_start(
                        out=oT_sb[:, pc, :, :],
                        in_=ot_o[0, :, :].bitcast(F8).rearrange(
                            "p (j t) -> p j t", j=2))
                    nc.sync.dma_start(
                        out=oT_sb[:, 2 + pc, :, :],
                        in_=ot_o[1, :, :].bitcast(F8).rearrange(
                            "p (j t) -> p j t", j=2))

        # ---- phase 7+8: proj (fp8 DoubleRow) -> attn rows bf16, with the
        # LN row-sum accumulated on the proj copies and a shortened
        # mean/var chain per tb ----
        attn_bf = act.tile([128, 4, D], BF, tag="attn")
        xin = act.tile([128, 4, D], BF, tag="xin")
        for tb in range(4):
            sm = sml.tile([128, 2], F32, tag="sm", name=f"sm{tb}")
            sq2 = sml.tile([128, 2], F32, tag="sq2", name=f"sq2{tb}")
            sqs = xsp.tile([128, 4, D], BF, tag="xr", name=f"sqs{tb}")
            for n in range(2):
                pp = mm.tile([128, 512], F32, tag="mm", name=f"pj{tb}_{n}")
                for gi, g in enumerate([0, 2, 1, 3]):
                    nc.tensor.matmul(
                        out=pp[:], lhsT=oT_sb[:, g, :, tb * 128:(tb + 1) * 128],
                        rhs=wproj_sb[:, g, :, n * 512:(n + 1) * 512],
                        start=(gi == 0), stop=(gi == 3), perf_mode=DR,
                    )
                half = attn_bf[:, tb, n * 512:(n + 1) * 512]
                if n == 0:
                    nc.vector.tensor_scalar(
                        out=half, in0=pp[:], scalar1=WSI, scalar2=0.0,
                        op0=OP.mult, op1=OP.add, accum_out=sm[:, n:n + 1])
                else:
                    nc.scalar.activation(
                        out=half, in_=pp[:], func=AF.Copy, scale=WSI,
                        accum_out=sm[:, n:n + 1])
                nc.vector.scalar_tensor_tensor(
                    out=sqs[:, n, 0:512], in0=half, scalar=0.0, in1=half,
                    op0=OP.add, op1=OP.mult, accum_out=sq2[:, n:n + 1],
                )
            at = attn_bf[:, tb, :]
            mu = sml.tile([128, 1], F32, tag="mu", name=f"mu{tb}")
            nc.vector.scalar_tensor_tensor(
                out=mu[:], in0=sm[:, 0:1], scalar=1.0, in1=sm[:, 1:2],
                op0=OP.mult, op1=OP.add)
            nc.vector.tensor_scalar_mul(mu[:], mu[:], 1.0 / D)
            mu2 = sml.tile([128, 1], F32, tag="mu2", name=f"mu2{tb}")
            nc.vector.tensor_scalar(
                out=mu2[:], in0=mu[:], scalar1=mu[:, 0:1], scalar2=-1e-5,
                op0=OP.mult, op1=OP.add)
            ssq = sml.tile([128, 1], F32, tag="ssq", name=f"ssq{tb}")
            nc.vector.scalar_tensor_tensor(
                out=ssq[:], in0=sq2[:, 0:1], scalar=1.0, in1=sq2[:, 1:2],
                op0=OP.mult, op1=OP.add)
            var = sml.tile([128, 1], F32, tag="var", name=f"var{tb}")
            nc.vector.scalar_tensor_tensor(
                out=var[:], in0=ssq[:], scalar=1.0 / D, in1=mu2[:],
                op0=OP.mult, op1=OP.subtract)
            sd = sml.tile([128, 1], F32, tag="sd", name=f"sd{tb}")
            nc.scalar.activation(out=sd[:], in_=var[:], func=AF.Sqrt)
            rr = sml.tile([128, 1], F32, tag="rr", name=f"rr{tb}")
            nc.vector.reciprocal(rr[:], sd[:])
            for nh in range(2):
                nc.vector.tensor_scalar(
                    out=xin[:, tb, nh * 512:(nh + 1) * 512],
                    in0=attn_bf[:, tb, nh * 512:(nh + 1) * 512],
                    scalar1=mu[:, :1], scalar2=rr[:, :1],
                    op0=OP.subtract, op1=OP.mult,
                )
        if debug:
            nc.sync.dma_start(out=attn_dbg[:, :, :], in_=attn_bf[:, :, :])

        # ---- phase 9: xiT fp8 [128, 4, 2, 512]; one [128,1024] PSUM
        # tile and one copy per g group ----
        xiT = act.tile([128, 4, 2, K], F8, tag="xiT")
        for g in range(4):
            xps = trp.tile([128, 1024], BF, tag="trp", name=f"xit{g}")
            for j in range(2):
                dc = g * 2 + j
                for tb in range(4):
                    nc.tensor.transpose(
                        out=xps[:, j * 512 + tb * 128:j * 512 + (tb + 1) * 128],
                        in_=xin[:, tb, dc * 128:(dc + 1) * 128],
                        identity=identb[:],
                    )
            if g % 2 == 0:
                nc.vector.tensor_copy(
                    out=xiT[:, g, :, :],
                    in_=xps[:].rearrange("p (j t) -> p j t", j=2))
            else:
                nc.scalar.activation(
                    out=xiT[:, g, :, :],
                    in_=xps[:].rearrange("p (j t) -> p j t", j=2),
                    func=AF.Copy)

        # ---- phase 10: fc + gelu (fp8 DoubleRow) ----
        hT = act.tile([128, 8, 2, K], F8, tag="hT")
        for f in range(16):
            fp = mm.tile([128, 512], F32, tag="mm", name=f"fc{f}")
            for g in range(4):
                nc.tensor.matmul(
                    out=fp[:], lhsT=wfc_sb[:, g, :, f * 128:(f + 1) * 128],
                    rhs=xiT[:, g, :, :],
                    start=(g == 0), stop=(g == 3), perf_mode=DR,
                )
            if not gelu_exact:
                hb = xsp.tile([128, 4, D], BF, tag="xr", name=f"hb{f}")
                nc.scalar.activation(out=hb[:, 0, 0:512], in_=fp[:],
                                     func=AF.Gelu_apprx_tanh, scale=WSI)
                heng = nc.vector if f % 2 == 0 else nc.gpsimd
                heng.tensor_copy(out=hT[:, f // 2, f % 2, :],
                                 in_=hb[:, 0, 0:512])
            else:
                # x*sigmoid(1.5957691*(x+0.044715*x^3)) via Exp+reciprocal
                hs = xsp.tile([128, 4, D], BF, tag="xr", name=f"gh{f}")
                x1 = hs[:, 0, 0:512]
                nc.vector.tensor_scalar_mul(x1, fp[:], WSI)
                h2 = hs[:, 1, 0:512]
                nc.vector.tensor_mul(out=h2, in0=x1, in1=x1)
                nc.vector.scalar_tensor_tensor(
                    out=h2, in0=h2, scalar=0.044715, in1=x1,
                    op0=OP.mult, op1=OP.mult,
                )
                nc.vector.tensor_add(out=h2, in0=h2, in1=x1)
                nc.scalar.activation(out=h2, in_=h2, func=AF.Exp,
                                     scale=-2.0 * 0.7978845608028654)
                nc.vector.tensor_scalar_add(h2, h2, 1.0)
                h3 = hs[:, 2, 0:512]
                with nc.allow_low_precision(reason="sigmoid denom, bf16 ok"):
                    nc.vector.reciprocal(h3, h2)
                nc.vector.tensor_mul(out=hT[:, f // 2, f % 2, :], in0=h3,
                                     in1=x1)

        # ---- phase 11: out partial (x64) -> updp bf16; per-half stores
        # so the final DMA tail is short, Act copy first / DVE last ----
        upd_sb = act.tile([128, 4, D], BF, tag="updp")
        for tb in range(4):
            for n in (1, 0):
                op_ps = mm.tile([128, 512], F32, tag="mm", name=f"ou{tb}_{n}")
                for g in range(8):
                    nc.tensor.matmul(
                        out=op_ps[:], lhsT=hT[:, g, :, tb * 128:(tb + 1) * 128],
                        rhs=wout_sb[:, g, :, n * 512:(n + 1) * 512],
                        start=(g == 0), stop=(g == 7), perf_mode=DR,
                    )
                nc.vector.tensor_copy(
                    out=upd_sb[:, tb, n * 512:(n + 1) * 512],
                    in_=op_ps[:])
                seng = nc.sync if n == 0 else nc.scalar
                seng.dma_start(out=updp[:, tb, n * 512:(n + 1) * 512],
                               in_=upd_sb[:, tb, n * 512:(n + 1) * 512])

    nc.compile()
    return nc


_CACHE = {}


def _get_program(n_cores=8):
    if n_cores not in _CACHE:
        _CACHE[n_cores] = build_program(n_cores)
    return _CACHE[n_cores]


def _prep_shared(inputs):
    """Host-side weight shuffles/casts (shared by all cores)."""
    w_router = np.asarray(inputs["w_router"], np.float32)
    w_qkv = np.asarray(inputs["w_qkv"], np.float32)
    w_proj = np.asarray(inputs["w_proj"], np.float32)
    w_fc = np.asarray(inputs["w_fc"], np.float32)
    w_out = np.asarray(inputs["w_out"], np.float32)

    w64 = w_router[:, 0].astype(np.float32) * WS
    w8 = w64.astype(F8NP).astype(np.float32)
    ratio = np.divide(w8, w64, out=np.zeros_like(w64), where=w64 != 0)
    # router weights in the standard DoubleRow k-map with 128 replicated
    # output columns: d = g*256 + j*128 + p
    w8dr = np.ascontiguousarray(np.broadcast_to(
        w8.reshape(4, 2, 128, 1).transpose(2, 0, 1, 3),
        (128, 4, 2, 32)).astype(F8NP))
    w64b = np.ascontiguousarray(np.broadcast_to(
        w64.reshape(8, 128).T[:, :, None], (128, 8, 32)).astype(BF16NP))
    identb = np.eye(128, dtype=BF16NP)
    iota16 = (np.arange(256)[None, :] * 16 + np.arange(16)[:, None]
              + 1).astype(np.float32)
    rep16 = np.zeros((16, 128), np.float32)
    for p in range(128):
        rep16[p % 16, p] = 1.0
    onesb = np.ones((128, 8), BF16NP)
    ar = np.arange(128)
    trilq = (ar[None, :] >= ar[:, None]).astype(BF16NP)

    p_ = np.arange(128)
    g_ = np.arange(4)
    j_ = np.arange(2)
    # gather-layout row map: tokT partition p, group g, sub j holds
    # x row d = 2*(g*128+p)+j
    dmap_gather = (2 * (g_[None, :, None] * 128 + p_[:, None, None])
                   + j_[None, None, :])
    wqkv_e = []
    for e in range(2):
        sl = np.concatenate(
            [w_qkv[:, o * D + e * 512:o * D + (e + 1) * 512]
             for o in range(3)], axis=1)
        wqkv_e.append(np.ascontiguousarray(
            (sl[dmap_gather.reshape(-1), :] * WS)
            .reshape(128, 4, 2, 3 * 512).astype(F8NP)))
    # standard DoubleRow k map: k row g*256 + j*128 + p
    dmap_std = (g_[None, :, None] * 256 + j_[None, None, :] * 128
                + p_[:, None, None])
    wproj_f8 = np.ascontiguousarray(
        (w_proj[dmap_std.reshape(-1), :] * WS).reshape(128, 4, 2, D)
        .astype(F8NP))
    halves = []
    g8 = np.arange(8)
    dmap8 = (g8[None, :, None] * 256 + j_[None, None, :] * 128
             + p_[:, None, None])
    for e in range(2):
        wfc_h = (w_fc[:, e * FCH:(e + 1) * FCH] * WS)
        wfc_f8 = np.ascontiguousarray(
            wfc_h[dmap_std.reshape(-1), :].reshape(128, 4, 2, FCH)
            .astype(F8NP))
        wout_h = (w_out[e * FCH:(e + 1) * FCH, :] * WS)
        wout_f8 = np.ascontiguousarray(
            wout_h[dmap8.reshape(-1), :].reshape(128, 8, 2, D).astype(F8NP))
        halves.append((wfc_f8, wout_f8))

    return dict(w8dr=w8dr, w64b=w64b, ratio=ratio,
                identb=identb, iota16=iota16,
                rep16=rep16, onesb=onesb, trilq=trilq,
                ones32=np.ones((1, 16), np.float32),
                wqkv_e=wqkv_e, wproj_f8=wproj_f8, halves=halves)


def make_in_maps(inputs, n_cores=8):
    x = np.asarray(inputs["x"], np.float32)
    sh = _prep_shared(inputs)
    xf8_all, x8s_all, xrT_all = [], {}, {}
    for b in range(B):
        xf8 = x[b].astype(F8NP)
        xrp = (x[b] - xf8.astype(np.float32) * sh["ratio"][None, :]
               ).astype(BF16NP)
        xf8_all.append(np.ascontiguousarray(xf8))
        for e in range(2):
            half8 = xf8[e * (S // 2):(e + 1) * (S // 2)]
            # x8s[q, p, g, j, s] = xf8[e*2048 + q*512 + s, g*256+j*128+p]
            x8s_all[b, e] = np.ascontiguousarray(
                half8.reshape(4, 512, 4, 2, 128).transpose(0, 4, 2, 3, 1))
            halfr = xrp[e * (S // 2):(e + 1) * (S // 2)]
            # xrT[q, p, blk, s] = r'[e*2048 + q*512 + s, blk*128+p]
            xrT_all[b, e] = np.ascontiguousarray(
                halfr.reshape(4, 512, 8, 128).transpose(0, 3, 2, 1))

    in_maps = []
    for c in range(n_cores):
        b, e = (c // 2) % B, c % 2
        wfc_f8, wout_f8 = sh["halves"][e]
        in_maps.append({
            "xf8": xf8_all[b],
            "x8s": x8s_all[b, e],
            "xrT": xrT_all[b, e],
            "w8dr": sh["w8dr"],
            "w64": sh["w64b"],
            "wqkv": sh["wqkv_e"][e],
            "wproj": sh["wproj_f8"],
            "wfc": wfc_f8,
            "wout": wout_f8,
            "identb": sh["identb"],
            "ones32": sh["ones32"],
            "iota16": sh["iota16"],
            "rep16": sh["rep16"],
            "onesb": sh["onesb"],
            "trilq": sh["trilq"],
        })
    return in_maps


def assemble_output(x, results):
    out = np.array(x, np.float32, copy=True)
    nb = len(results) // 2
    for b in range(nb):
        r0, r1 = results[2 * b], results[2 * b + 1]
        for r in (r0, r1):
            nf = int(np.asarray(r["nf_out"]).reshape(-1)[0])
            assert nf == K, f"batch {b}: expected {K} selected, got {nf}"
        pos = np.asarray(r0["pos_out"]).T.reshape(-1)     # [512] slot order
        u0 = np.asarray(r0["updp"]).astype(np.float32)    # [128, 4, 1024]
        u1 = np.asarray(r1["updp"]).astype(np.float32)
        part = (u0 + u1) * WSI
        rows = part.transpose(1, 0, 2).reshape(K, D)      # row s = tb*128+p
        out[b, pos] += rows
    return out


def kernel(**inputs):
    nc = _get_program(8)
    in_maps = make_in_maps(inputs, 8)
    res = run_bass_kernel_spmd(nc, in_maps, list(range(8))).results
    x = np.asarray(inputs["x"], np.float32)
    return assemble_output(x, res)


if __name__ == "__main__":
    nc = build_program(8)
    print("program built + compiled OK")

